# revision 22
# baseline (speedup 1.0000x reference)
"""Trainium2 Bass kernel for CausalWanSelfAttention (block-causal window attention).

Geometry: B=1, S=6240, DIM=1536, H=12 heads x D=128, frames of L=1560 tokens,
window = current + previous frame.

Sharding over 8 NeuronCores (sequence-parallel with KV AllGather):
  - core c owns tokens [780c, 780c+780): computes fused QKV for them
    (weights replicated), full-dim RMSNorm + RoPE locally,
  - AllGathers normed/roped K (feature-major [1536,780]) and V
    (token-major [780,1536]) across cores in bf16,
  - attends its 780 queries to its 2-frame KV window (3120 tokens) read from
    the gathered buffers at per-core dynamic offsets. Frame-0 cores use a
    duplicated-frame window (softmax over a duplicated key set equals softmax
    over the single set exactly), so no masking is needed anywhere,
  - local output projection (all heads of a token live on one core).

Layouts: q,k are feature-major [d, token]; v is token-major [token, d] so it
can be the stationary operand of the PV matmul directly. The head-dim order
of q,k is de-interleaved on the host (even rotary lanes first, odd second)
so RoPE works on contiguous partition halves.

PE-efficiency notes (vs the first working version):
  - no K=1/M=1 broadcast matmuls: the RMSNorm 1/rms is computed 128-row
    replicated (ones[128,128] stationary costs the same as ones[128,1]) and
    folded into per-path RoPE cos/sin tables; the norm gain g and its bias
    are folded into the PSUM-evacuation activation (per-partition scale/bias),
  - softmax denominators are accumulated 128-replicated and inverted on the
    vector engine, then applied as a plain elementwise multiply,
  - one flat pool scope: attention head h starts as soon as q head h is
    roped, overlapping the q-path rope tail and the kv window DMAs with
    score/PV matmuls,
  - per-chunk interleave (scores -> exp -> den+PV accumulate) keeps only 4
    prob tiles live instead of 28 and pairs matmuls on shared stationaries.

Precision: matmul operands bf16 (fp32 PSUM accumulation); RMSNorm statistics
fp32; RoPE applied in bf16; softmax normalization fp32.
"""

from contextlib import ExitStack

import ml_dtypes
import numpy as np

import concourse.bass as bass
import concourse.bacc as bacc
import concourse.mybir as mybir
import concourse.tile as tile
from concourse import bass_utils

F32 = mybir.dt.float32
BF16 = mybir.dt.bfloat16
U32 = mybir.dt.uint32
AF = mybir.ActivationFunctionType
ALU = mybir.AluOpType
NP_BF16 = ml_dtypes.bfloat16

# Geometry (hardcoded per the problem spec).
S, DIM, H, D = 6240, 1536, 12, 128
HD = H * D                      # 1536
L = 1560                        # frame length
NCORES = 8
T = S // NCORES                 # 780 tokens per core
QG = 390                        # query/token group: 2 per core, fits one PSUM bank
EPS = 1e-6
KQ = DIM // 128                 # 12 contraction chunks for the QKV matmuls
# token sub-tiles within a 780-token rank block: 6x128 + 1x12
TOK_SPLITS = [(i * 128, min(128, T - i * 128)) for i in range((T + 127) // 128)]
N_KC = 25                       # 3120-key window in 128-key chunks (24x128+48)


def _build_nc():
    nc = bacc.Bacc("TRN2", target_bir_lowering=False, debug=False,
                   enable_asserts=True, num_devices=NCORES)

    # ---- per-core inputs ----
    hidT = nc.dram_tensor("hidT", [DIM + 1, T], BF16, kind="ExternalInput").ap()
    csd = nc.dram_tensor("csd", [128, 2 * T], F32, kind="ExternalInput").ap()
    wink = nc.dram_tensor("wink", [1, 4], U32, kind="ExternalInput").ap()  # 1536*w
    winv = nc.dram_tensor("winv", [1, 4], U32, kind="ExternalInput").ap()  # 780*w

    # ---- replicated inputs ----
    WqkT = nc.dram_tensor("WqkT", [DIM, 2 * HD], BF16, kind="ExternalInput").ap()
    WvTa = nc.dram_tensor("WvTa", [DIM + 1, HD], BF16, kind="ExternalInput").ap()
    bqk = nc.dram_tensor("bqk", [128, 2 * H], F32, kind="ExternalInput").ap()
    gqk = nc.dram_tensor("gqk", [128, 2 * H], F32, kind="ExternalInput").ap()
    bgqk = nc.dram_tensor("bgqk", [128, 2 * H], F32, kind="ExternalInput").ap()
    WoT = nc.dram_tensor("WoT", [HD, DIM], BF16, kind="ExternalInput").ap()
    bo = nc.dram_tensor("bo", [128, DIM // 128], F32, kind="ExternalInput").ap()

    # ---- output (feature-major; host transposes back) ----
    outT = nc.dram_tensor("outT", [DIM, T], F32, kind="ExternalOutput").ap()

    # ---- internal DRAM for the collectives (split <1MB per rank: mesh algo,
    # pipelined so attention heads unlock progressively) ----
    kcon = [nc.dram_tensor(f"kcon{g}", [3 * 128, T], BF16) for g in range(4)]
    vcon = [nc.dram_tensor(f"vcon{o}", [T, 512], BF16) for o in range(3)]
    gk = [nc.dram_tensor(f"gk{g}", [NCORES * 3 * 128, T], BF16,
                         addr_space="Shared") for g in range(4)]
    gv = [nc.dram_tensor(f"gv{o}", [NCORES * T, 512], BF16,
                         addr_space="Shared") for o in range(3)]

    with tile.TileContext(nc) as tc:
        _emit(nc, tc, hidT, csd, wink, winv, WqkT, WvTa, bqk, gqk, bgqk,
              WoT, bo, outT, kcon, vcon, gk, gv)
    nc.compile()
    return nc


def _emit(nc, tc, hidT, csd, wink, winv, WqkT, WvTa, bqk, gqk, bgqk,
          WoT, bo, outT, kcon, vcon, gk, gv):
    # window base registers (element offsets into gk / gv axis 0)
    kregs, vregs = [], []
    for i in range(4):
        rk = nc.alloc_registers(f"wk{i}")
        nc.regs_load(rk, wink.tensor[0:1, i:i + 1])
        kregs.append(nc.snap(rk, donate=True, min_val=0,
                             max_val=(NCORES - 1) * 3 * 128))
        rv = nc.alloc_registers(f"wv{i}")
        nc.regs_load(rv, winv.tensor[0:1, i:i + 1])
        vregs.append(nc.snap(rv, donate=True, min_val=0,
                             max_val=(NCORES - 1) * T))

    GS = (slice(0, QG), slice(QG, 2 * QG))        # token groups in SBUF
    PS2 = (slice(0, QG), slice(512, 512 + QG))    # the two bank-aligned halves

    def act2(out_sb, ps2, func, bias=0.0, scale=1.0):
        """One ACT op over both 390-wide halves of a 2-bank PSUM tile."""
        nc.scalar.activation(
            out_sb.rearrange("p (a b) -> p a b", a=2),
            ps2.rearrange("p (a b) -> p a b", a=2)[:, :, 0:QG],
            func, bias=bias, scale=scale)

    with ExitStack() as ctx:
        pool = lambda **kw: ctx.enter_context(tc.tile_pool(**kw))
        const = pool(name="const", bufs=1)
        hid_pool = pool(name="hid", bufs=1)
        cs_pool = pool(name="csp", bufs=1)
        csq_pool = pool(name="csq", bufs=2)
        wl_pool = pool(name="wls", bufs=3)
        vw_pool = pool(name="vws", bufs=1)
        wrk_pool = pool(name="wrk", bufs=1)
        tmp_pool = pool(name="tmp", bufs=2)
        rope_pool = pool(name="ropet", bufs=2)
        small_pool = pool(name="small", bufs=1)
        q_pool = pool(name="qsb", bufs=1)        # roped q (bf16)
        att_pool = pool(name="attsb", bufs=1)    # k (early) + attn out
        kv_pool = pool(name="kwin", bufs=2)
        vt_pool = pool(name="vwin", bufs=2)
        probs_pool = pool(name="probs", bufs=4)
        attm_pool = pool(name="attm", bufs=2)
        wo_pool = pool(name="wos", bufs=3)
        o_pool = pool(name="osbp", bufs=2)
        # PSUM: 4 + 2 + 2 = 8 banks
        big_ps = pool(name="bigps", bufs=2, space="PSUM")
        op_ps = pool(name="opps", bufs=1, space="PSUM")
        red_ps = pool(name="redden", bufs=1, space="PSUM")

        # ---- input DMAs: hid first (feeds the first matmuls) ----
        hid = [hid_pool.tile([128, T], BF16, tag=f"hid{i}", name=f"hid{i}")
               for i in range(KQ)]
        for i in range(KQ):
            nc.sync.dma_start(hid[i], hidT.tensor[128 * i:128 * (i + 1), :])
        hid_ones = hid_pool.tile([1, T], BF16, tag="hid_ones")
        nc.sync.dma_start(hid_ones, hidT.tensor[DIM:DIM + 1, :])
        # [cos;cos] in cols 0:T, [-sin;sin] in cols T:2T (gpsimd queue: keeps
        # the sync queue clear for hid + first weight tiles)
        cs_sb = cs_pool.tile([128, 2 * T], F32)
        nc.gpsimd.dma_start(cs_sb, csd)

        ones_col = const.tile([128, 128], F32)        # fp32 ones (norm reduce)
        nc.vector.memset(ones_col, 1.0)
        ones_bf = const.tile([128, 128], BF16)        # bf16 ones (denominator)
        nc.vector.memset(ones_bf, 1.0)
        bqk_sb = const.tile([128, 2 * H], F32)
        nc.sync.dma_start(bqk_sb, bqk)
        gqk_sb = const.tile([128, 2 * H], F32)
        nc.sync.dma_start(gqk_sb, gqk)
        bgqk_sb = const.tile([128, 2 * H], F32)
        nc.sync.dma_start(bgqk_sb, bgqk)
        bo_sb = const.tile([128, DIM // 128], F32)
        nc.sync.dma_start(bo_sb, bo)
        eps_q = const.tile([128, 1], F32)
        nc.vector.memset(eps_q, D * EPS)
        eps_k = const.tile([128, 1], F32)
        nc.vector.memset(eps_k, EPS)

        # ================= QKV projections, norms, rope, gathers =============
        def qk_path(which, dest_tiles, chunk_done=None):
            mlo = H if which == "k" else 0
            # --- projection + g-folded evac + sum of squares ---
            ssq = small_pool.tile([128, T], F32, tag="ssq")
            works = []
            for mi in range(H):
                m = mlo + mi
                work = wrk_pool.tile([128, T], BF16, tag=f"work{mi}",
                                     name=f"work{mi}")
                works.append(work)
                tsq = tmp_pool.tile([128, T], F32, tag="tsq")
                ps2 = big_ps.tile([128, 1024], F32, tag="qkps")
                w_sb = wl_pool.tile([128, KQ * 128], BF16, tag="wqk")
                nc.sync.dma_start(
                    w_sb.rearrange("p (c m) -> p c m", c=KQ),
                    WqkT.tensor[:, 128 * m:128 * (m + 1)].rearrange(
                        "(c p) m -> p c m", p=128))
                for kc in range(KQ):
                    for g in range(2):
                        nc.tensor.matmul(ps2[:, PS2[g]],
                                         w_sb[:, 128 * kc:128 * (kc + 1)],
                                         hid[kc][:, GS[g]],
                                         start=(kc == 0),
                                         stop=(kc == KQ - 1))
                b = bqk_sb[:, m:m + 1]
                # work = g*(x+b) in bf16; squares from (x+b) in fp32
                act2(work, ps2, AF.Identity, bias=bgqk_sb[:, m:m + 1],
                     scale=gqk_sb[:, m:m + 1])
                act2(tsq, ps2, AF.Square, bias=b)
                if mi == 0:
                    nc.vector.tensor_copy(ssq, tsq)
                else:
                    nc.vector.tensor_tensor(ssq, ssq, tsq, ALU.add)
            # --- rms scale, 128-replicated: inv = 1/sqrt(mean+eps) (x 1/sqrt(D)
            # for q), then folded into the rope tables ---
            sq_scale = (D / DIM) if which == "q" else (1.0 / DIM)
            sq_bias = eps_q if which == "q" else eps_k
            inv = small_pool.tile([128, T], F32, tag="inv")
            rt = small_pool.tile([128, T], F32, tag="rt")
            for g in range(2):
                red = red_ps.tile([128, QG], F32, tag=f"dp{g}", name=f"dp{g}")
                nc.tensor.matmul(red, ones_col, ssq[:, GS[g]], start=True,
                                 stop=True)
                nc.scalar.activation(rt[:, GS[g]], red, AF.Sqrt,
                                     bias=sq_bias, scale=sq_scale)
            nc.vector.reciprocal_approx_fast(inv, rt)
            cq = csq_pool.tile([128, T], BF16, tag="cq")
            sq = csq_pool.tile([128, T], BF16, tag="sq")
            nc.vector.tensor_tensor(cq, cs_sb[:, 0:T], inv, ALU.mult)
            nc.vector.tensor_tensor(sq, cs_sb[:, T:2 * T], inv, ALU.mult)
            # --- rope -> bf16 dest, per head chunk ---
            for mi in range(H):
                work = works[mi]
                dest = dest_tiles[mi]
                sw = rope_pool.tile([128, T], BF16, tag="rsw")
                nc.gpsimd.dma_start(sw[0:64, :], work[64:128, :])
                nc.gpsimd.dma_start(sw[64:128, :], work[0:64, :])
                for g in range(2):
                    qs = GS[g]
                    ta = rope_pool.tile([128, QG], BF16, tag="ra")
                    nc.vector.tensor_tensor(ta, work[:, qs], cq[:, qs],
                                            ALU.mult)
                    tb = rope_pool.tile([128, QG], BF16, tag="rb")
                    nc.vector.tensor_tensor(tb, sw[:, qs], sq[:, qs],
                                            ALU.mult)
                    nc.vector.tensor_tensor(dest[:, qs], ta, tb, ALU.add)
                if chunk_done is not None:
                    chunk_done(mi, dest)

        # ---- v: token-major, contraction over dim chunks + bias row ----
        def emit_v_load(og):
            vb = small_pool.tile([1, 512], BF16, tag="vb")
            nc.sync.dma_start(
                vb, WvTa.tensor[DIM:DIM + 1, 512 * og:512 * (og + 1)])
            vw = [vw_pool.tile([128, 512], BF16, tag=f"vw{kc}",
                               name=f"vw{kc}") for kc in range(KQ)]
            for kc in range(KQ):
                nc.sync.dma_start(
                    vw[kc], WvTa.tensor[128 * kc:128 * (kc + 1),
                                        512 * og:512 * (og + 1)])
            return vb, vw

        def emit_v_compute(og, vb, vw):
            for ti, (t0, tn_) in enumerate(TOK_SPLITS):
                # alternate the two norm-reduce banks: double-buffers the V
                # chains without widening the PSUM budget
                ps = red_ps.tile([128, 512], F32, tag=f"dp{ti % 2}")
                for kc in range(KQ):
                    nc.tensor.matmul(ps[0:tn_, :],
                                     hid[kc][:, t0:t0 + tn_],
                                     vw[kc], start=(kc == 0), stop=False)
                nc.tensor.matmul(ps[0:tn_, :], hid_ones[:, t0:t0 + tn_],
                                 vb, start=False, stop=True)
                vsb = tmp_pool.tile([128, 512], BF16, tag="vsb")
                nc.scalar.activation(vsb[0:tn_, :], ps[0:tn_, :],
                                     AF.Identity)
                # scalar queue: keeps this dependent store off the sync queue
                # (in-order queues head-of-line block later independent loads)
                nc.scalar.dma_start(vcon[og].ap()[t0:t0 + tn_, :],
                                    vsb[0:tn_, :])
            nc.gpsimd.collective_compute(
                "AllGather", ALU.bypass,
                replica_groups=[list(range(NCORES))],
                ins=[vcon[og].ap()], outs=[gv[og].ap()])

        # V first: its AllGathers must not stall attention heads 4/8; og=2
        # is deferred so its matmuls fill the k-path rope PE window.
        emit_v_compute(0, *emit_v_load(0))
        emit_v_compute(1, *emit_v_load(1))

        # ---- k (feeds the k collectives, 3 heads per chunk) ----
        k_tiles = [att_pool.tile([128, T], BF16, tag=f"att{h}",
                                 name=f"kt{h}") for h in range(H)]

        def k_chunk_done(mi, dest):
            g = mi // 3
            nc.scalar.dma_start(
                kcon[g].ap()[128 * (mi % 3):128 * (mi % 3 + 1), :], dest)
            if mi % 3 == 2:
                nc.gpsimd.collective_compute(
                    "AllGather", ALU.bypass,
                    replica_groups=[list(range(NCORES))],
                    ins=[kcon[g].ap()], outs=[gk[g].ap()])

        qk_path("k", k_tiles, k_chunk_done)

        emit_v_compute(2, *emit_v_load(2))

        # ---- attention kv window prefetch (2 heads deep) ----
        def kv_prefetch(h):
            ksb = kv_pool.tile([128, 4 * T], BF16, tag="ksb")
            for w in range(4):
                nc.gpsimd.dma_start(
                    ksb[:, w * T:(w + 1) * T],
                    gk[h // 3][bass.ds(kregs[w] + 128 * (h % 3), 128), :])
            ho = 128 * (h % 4)
            vwin = vt_pool.tile([128, 25 * 128], BF16, tag="vwin")
            for w in range(4):
                lo = 780 * w          # window-space start of this block
                s = lo
                while s < lo + 780:
                    off = s % 128
                    if off:
                        n = min(128 - off, lo + 780 - s)
                    else:
                        n = lo + 780 - s
                    blk = s // 128
                    if off == 0 and n >= 128:
                        nb = n // 128
                        nc.gpsimd.dma_start(
                            vwin[:, 128 * blk:128 * (blk + nb)].rearrange(
                                "p (c d) -> p c d", d=128),
                            gv[h // 4][bass.ds(vregs[w] + (s - lo),
                                               128 * nb),
                                       ho:ho + 128].rearrange(
                                           "(c p) d -> p c d", p=128))
                        s += 128 * nb
                    else:
                        n = min(n, 128 - off)
                        nc.gpsimd.dma_start(
                            vwin[off:off + n,
                                 128 * blk:128 * (blk + 1)],
                            gv[h // 4][bass.ds(vregs[w] + (s - lo), n),
                                       ho:ho + 128])
                        s += n
            return ksb, vwin

        # issue heads 0/1 before the q path so attention can start the moment
        # q head 0 is roped (their DMAs depend only on the k/v collectives)
        pref = [kv_prefetch(0), kv_prefetch(1)]

        # ---- q ----
        q_tiles = [q_pool.tile([128, T], BF16, tag=f"q{h}", name=f"qt{h}")
                   for h in range(H)]
        qk_path("q", q_tiles)

        # ================= attention =========================================
        att_tiles = []
        for h in range(H):
            ksb, vwin = pref[h]
            ath = att_pool.tile([128, T], BF16, tag=f"att{h}")
            att_tiles.append(ath)
            op2 = op_ps.tile([128, 1024], F32, tag="op")
            dps = [red_ps.tile([128, QG], F32, tag="dp0", name="dp0"),
                   red_ps.tile([128, QG], F32, tag="dp1", name="dp1")]
            # per-chunk interleave: scores -> exp -> den+PV accumulate
            for ci in range(N_KC):
                c0 = 128 * ci
                cn = min(128, 4 * T - c0)          # window is 3120 tokens
                sp2 = big_ps.tile([128, 1024], F32, tag="qkps")
                for g in range(2):
                    nc.tensor.matmul(
                        sp2[0:cn, PS2[g]], ksb[:, c0:c0 + cn],
                        q_tiles[h][:, GS[g]], start=True, stop=True)
                pr = probs_pool.tile([128, 2 * QG], BF16, tag="pr")
                act2(pr[0:cn, :], sp2[0:cn, :], AF.Exp)
                for g in range(2):
                    nc.tensor.matmul(dps[g], ones_bf[0:cn, :],
                                     pr[0:cn, GS[g]],
                                     start=(ci == 0), stop=(ci == N_KC - 1))
                vt = vwin[:, 128 * ci:128 * (ci + 1)]
                for g in range(2):
                    nc.tensor.matmul(op2[:, PS2[g]], vt[0:cn, :],
                                     pr[0:cn, GS[g]],
                                     start=(ci == 0), stop=(ci == N_KC - 1))
            osb = attm_pool.tile([128, 2 * QG], F32, tag="osb")
            act2(osb, op2, AF.Identity)
            dsb = attm_pool.tile([128, 2 * QG], F32, tag="dsb")
            for g in range(2):
                nc.vector.reciprocal_approx_fast(dsb[:, GS[g]], dps[g])
            nc.vector.tensor_tensor(ath, osb, dsb, ALU.mult)
            if h + 2 < H:
                pref.append(kv_prefetch(h + 2))

        # ================= output projection =================================
        for od in range(DIM // 128):
            wo = wo_pool.tile([128, HD], BF16, tag="wo")
            nc.sync.dma_start(
                wo.rearrange("p (c m) -> p c m", c=H),
                WoT.tensor[:, 128 * od:128 * (od + 1)].rearrange(
                    "(c p) m -> p c m", p=128))
            ot = o_pool.tile([128, T], F32, tag="ot")
            ps2 = big_ps.tile([128, 1024], F32, tag="qkps")
            for hc in range(H):
                for g in range(2):
                    nc.tensor.matmul(ps2[:, PS2[g]],
                                     wo[:, 128 * hc:128 * (hc + 1)],
                                     att_tiles[hc][:, GS[g]],
                                     start=(hc == 0), stop=(hc == H - 1))
            act2(ot, ps2, AF.Identity, bias=bo_sb[:, od:od + 1])
            nc.scalar.dma_start(outT.tensor[128 * od:128 * (od + 1), :], ot)


_CACHED_NC = None
_LAST_IN_MAPS = None


def _get_nc():
    global _CACHED_NC
    if _CACHED_NC is None:
        _CACHED_NC = _build_nc()
    return _CACHED_NC


def _deinterleave(n):
    """Permutation putting even rotary lanes first within each 128-dim head."""
    idx = np.arange(n).reshape(-1, D)
    return np.concatenate([idx[:, 0::2], idx[:, 1::2]], axis=1).reshape(-1)


def kernel(hidden_states, freqs_cos, freqs_sin, W_qkv, b_qkv, gq, gk, W_out,
           b_out):
    hidden_states = np.asarray(hidden_states, dtype=np.float32)
    freqs_cos = np.asarray(freqs_cos, dtype=np.float32)
    freqs_sin = np.asarray(freqs_sin, dtype=np.float32)
    W_qkv = np.asarray(W_qkv, dtype=np.float32)
    b_qkv = np.asarray(b_qkv, dtype=np.float32)
    gq = np.asarray(gq, dtype=np.float32)
    gk = np.asarray(gk, dtype=np.float32)
    W_out = np.asarray(W_out, dtype=np.float32)
    b_out = np.asarray(b_out, dtype=np.float32)

    nc = _get_nc()

    perm = _deinterleave(HD)
    Wq, Wk, Wv = W_qkv[:HD][perm], W_qkv[HD:2 * HD][perm], W_qkv[2 * HD:]
    bq, bk, bv = b_qkv[:HD][perm], b_qkv[HD:2 * HD][perm], b_qkv[2 * HD:]
    gqp, gkp = gq[perm], gk[perm]

    WqkT = np.ascontiguousarray(
        np.concatenate([Wq, Wk], axis=0).T).astype(NP_BF16)   # [1536, 3072]
    WvTa = np.concatenate([Wv.T, bv[None, :]],
                          axis=0).astype(NP_BF16)             # [1537, 1536]
    bcat = np.concatenate([bq, bk])
    gcat = np.concatenate([gqp, gkp])
    bqk_t = np.ascontiguousarray(bcat.reshape(2 * H, 128).T)  # [128, 24]
    gqk_t = np.ascontiguousarray(gcat.reshape(2 * H, 128).T)
    bgqk_t = np.ascontiguousarray((bcat * gcat).reshape(2 * H, 128).T)
    WoT = np.ascontiguousarray(W_out.T).astype(NP_BF16)       # [1536, 1536]
    bo = np.ascontiguousarray(b_out.reshape(DIM // 128, 128).T)  # [128, 12]

    in_maps = []
    for c in range(NCORES):
        sl = slice(c * T, (c + 1) * T)
        hidT = np.concatenate([
            np.ascontiguousarray(hidden_states[0, sl, :].T),
            np.ones((1, T), np.float32)], axis=0).astype(NP_BF16)  # [1537, 780]
        f = (c * T) // L
        if f == 0:
            win = [0, 1, 0, 1]
        else:
            base = 2 * (f - 1)
            win = [base, base + 1, base + 2, base + 3]
        cc = np.ascontiguousarray(freqs_cos[sl].T)            # [64, 780]
        ss = np.ascontiguousarray(freqs_sin[sl].T)
        csd = np.concatenate([
            np.concatenate([cc, cc], axis=0),
            np.concatenate([-ss, ss], axis=0)], axis=1)       # [128, 1560]
        in_maps.append({
            "hidT": hidT,
            "csd": csd,
            "wink": np.asarray([[w * 3 * 128 for w in win]], np.uint32),
            "winv": np.asarray([[w * T for w in win]], np.uint32),
            "WqkT": WqkT, "WvTa": WvTa, "bqk": bqk_t, "gqk": gqk_t,
            "bgqk": bgqk_t, "WoT": WoT, "bo": bo,
        })

    global _LAST_IN_MAPS
    _LAST_IN_MAPS = in_maps
    res = bass_utils.run_bass_kernel_spmd(nc, in_maps,
                                          core_ids=list(range(NCORES)))
    out = np.empty((1, S, DIM), np.float32)
    for c in range(NCORES):
        out[0, c * T:(c + 1) * T, :] = res.results[c]["outT"].T
    return out


# revision 30
# speedup vs baseline: 1.0174x; 1.0174x over previous
"""Trainium2 Bass kernel for CausalWanSelfAttention (block-causal window attention).

Geometry: B=1, S=6240, DIM=1536, H=12 heads x D=128, frames of L=1560 tokens,
window = current + previous frame.

Sharding over 8 NeuronCores (sequence-parallel with KV AllGather):
  - core c owns tokens [780c, 780c+780): computes fused QKV for them
    (weights replicated), full-dim RMSNorm + RoPE locally,
  - AllGathers normed/roped K (feature-major [1536,780]) and V
    (token-major [780,1536]) across cores in bf16,
  - attends its 780 queries to its 2-frame KV window (3120 tokens) read from
    the gathered buffers at per-core dynamic offsets. Frame-0 cores use a
    duplicated-frame window (softmax over a duplicated key set equals softmax
    over the single set exactly), so no masking is needed anywhere,
  - local output projection (all heads of a token live on one core).

Layouts: q,k are feature-major [d, token]; v is token-major [token, d] so it
can be the stationary operand of the PV matmul directly. The head-dim order
of q,k is de-interleaved on the host (even rotary lanes first, odd second)
so RoPE works on contiguous partition halves.

PE-efficiency notes (vs the first working version):
  - no K=1/M=1 broadcast matmuls: the RMSNorm 1/rms is computed 128-row
    replicated (ones[128,128] stationary costs the same as ones[128,1]) and
    folded into per-path RoPE cos/sin tables; the norm gain g and its bias
    are folded into the PSUM-evacuation activation (per-partition scale/bias),
  - softmax denominators are accumulated 128-replicated and inverted on the
    vector engine, then applied as a plain elementwise multiply,
  - one flat pool scope: attention head h starts as soon as q head h is
    roped, overlapping the q-path rope tail and the kv window DMAs with
    score/PV matmuls,
  - per-chunk interleave (scores -> exp -> den+PV accumulate) keeps only 4
    prob tiles live instead of 28 and pairs matmuls on shared stationaries.

Precision: matmul operands bf16 (fp32 PSUM accumulation); RMSNorm statistics
fp32; RoPE applied in bf16; softmax normalization fp32.
"""

from contextlib import ExitStack

import ml_dtypes
import numpy as np

import concourse.bass as bass
import concourse.bacc as bacc
import concourse.mybir as mybir
import concourse.tile as tile
from concourse import bass_utils

F32 = mybir.dt.float32
BF16 = mybir.dt.bfloat16
U32 = mybir.dt.uint32
AF = mybir.ActivationFunctionType
ALU = mybir.AluOpType
NP_BF16 = ml_dtypes.bfloat16

# Geometry (hardcoded per the problem spec).
S, DIM, H, D = 6240, 1536, 12, 128
HD = H * D                      # 1536
L = 1560                        # frame length
NCORES = 8
T = S // NCORES                 # 780 tokens per core
QG = 390                        # query/token group: 2 per core, fits one PSUM bank
EPS = 1e-6
KQ = DIM // 128                 # 12 contraction chunks for the QKV matmuls
# token sub-tiles within a 780-token rank block: 6x128 + 1x12
TOK_SPLITS = [(i * 128, min(128, T - i * 128)) for i in range((T + 127) // 128)]
N_KC = 25                       # 3120-key window in 128-key chunks (24x128+48)


def _build_nc():
    nc = bacc.Bacc("TRN2", target_bir_lowering=False, debug=False,
                   enable_asserts=True, num_devices=NCORES)

    # ---- per-core inputs ----
    hidT = nc.dram_tensor("hidT", [DIM + 1, T], BF16, kind="ExternalInput").ap()
    csd = nc.dram_tensor("csd", [128, 2 * T], F32, kind="ExternalInput").ap()
    wink = nc.dram_tensor("wink", [1, 4], U32, kind="ExternalInput").ap()  # 1536*w
    winv = nc.dram_tensor("winv", [1, 4], U32, kind="ExternalInput").ap()  # 780*w

    # ---- replicated inputs ----
    WqkT = nc.dram_tensor("WqkT", [DIM, 2 * HD], BF16, kind="ExternalInput").ap()
    WvTa = nc.dram_tensor("WvTa", [DIM + 1, HD], BF16, kind="ExternalInput").ap()
    bqk = nc.dram_tensor("bqk", [128, 2 * H], F32, kind="ExternalInput").ap()
    gqk = nc.dram_tensor("gqk", [128, 2 * H], F32, kind="ExternalInput").ap()
    bgqk = nc.dram_tensor("bgqk", [128, 2 * H], F32, kind="ExternalInput").ap()
    WoT = nc.dram_tensor("WoT", [HD, DIM], BF16, kind="ExternalInput").ap()
    bo = nc.dram_tensor("bo", [128, DIM // 128], F32, kind="ExternalInput").ap()

    # ---- output (feature-major; host transposes back) ----
    outT = nc.dram_tensor("outT", [DIM, T], F32, kind="ExternalOutput").ap()

    # ---- internal DRAM for the collectives (split <1MB per rank: mesh algo,
    # pipelined so attention heads unlock progressively) ----
    kcon = [nc.dram_tensor(f"kcon{g}", [128, T], BF16) for g in range(H)]
    vcon = [nc.dram_tensor(f"vcon{o}", [T, 512], BF16) for o in range(3)]
    gk = [nc.dram_tensor(f"gk{g}", [NCORES * 128, T], BF16,
                         addr_space="Shared") for g in range(H)]
    gv = [nc.dram_tensor(f"gv{o}", [NCORES * T, 512], BF16,
                         addr_space="Shared") for o in range(3)]

    with tile.TileContext(nc) as tc:
        _emit(nc, tc, hidT, csd, wink, winv, WqkT, WvTa, bqk, gqk, bgqk,
              WoT, bo, outT, kcon, vcon, gk, gv)
    nc.compile()
    return nc


def _emit(nc, tc, hidT, csd, wink, winv, WqkT, WvTa, bqk, gqk, bgqk,
          WoT, bo, outT, kcon, vcon, gk, gv):
    # window base registers (element offsets into gk / gv axis 0)
    kregs, vregs = [], []
    for i in range(4):
        rk = nc.alloc_registers(f"wk{i}")
        nc.regs_load(rk, wink.tensor[0:1, i:i + 1])
        kregs.append(nc.snap(rk, donate=True, min_val=0,
                             max_val=(NCORES - 1) * 128))
        rv = nc.alloc_registers(f"wv{i}")
        nc.regs_load(rv, winv.tensor[0:1, i:i + 1])
        vregs.append(nc.snap(rv, donate=True, min_val=0,
                             max_val=(NCORES - 1) * T))

    GS = (slice(0, QG), slice(QG, 2 * QG))        # token groups in SBUF
    PS2 = (slice(0, QG), slice(512, 512 + QG))    # the two bank-aligned halves

    def act2(out_sb, ps2, func, bias=0.0, scale=1.0):
        """One ACT op over both 390-wide halves of a 2-bank PSUM tile."""
        nc.scalar.activation(
            out_sb.rearrange("p (a b) -> p a b", a=2),
            ps2.rearrange("p (a b) -> p a b", a=2)[:, :, 0:QG],
            func, bias=bias, scale=scale)

    with ExitStack() as ctx:
        pool = lambda **kw: ctx.enter_context(tc.tile_pool(**kw))
        const = pool(name="const", bufs=1)
        hid_pool = pool(name="hid", bufs=1)
        cs_pool = pool(name="csp", bufs=1)
        csq_pool = pool(name="csq", bufs=2)
        wl_pool = pool(name="wls", bufs=3)
        vw_pool = pool(name="vws", bufs=1)
        wrk_pool = pool(name="wrk", bufs=1)
        tmp_pool = pool(name="tmp", bufs=2)
        rope_pool = pool(name="ropet", bufs=2)
        small_pool = pool(name="small", bufs=1)
        q_pool = pool(name="qsb", bufs=1)        # roped q (bf16)
        att_pool = pool(name="attsb", bufs=1)    # k (early) + attn out
        kv_pool = pool(name="kwin", bufs=2)
        vt_pool = pool(name="vwin", bufs=2)
        probs_pool = pool(name="probs", bufs=4)
        attm_pool = pool(name="attm", bufs=2)
        wo_pool = pool(name="wos", bufs=3)
        o_pool = pool(name="osbp", bufs=2)
        # PSUM: 4 + 2 + 2 = 8 banks
        big_ps = pool(name="bigps", bufs=2, space="PSUM")
        op_ps = pool(name="opps", bufs=1, space="PSUM")
        red_ps = pool(name="redden", bufs=1, space="PSUM")

        # ---- input DMAs: hid first (feeds the first matmuls) ----
        hid = [hid_pool.tile([128, T], BF16, tag=f"hid{i}", name=f"hid{i}")
               for i in range(KQ)]
        for i in range(KQ):
            nc.sync.dma_start(hid[i], hidT.tensor[128 * i:128 * (i + 1), :])
        hid_ones = hid_pool.tile([1, T], BF16, tag="hid_ones")
        nc.sync.dma_start(hid_ones, hidT.tensor[DIM:DIM + 1, :])
        # [cos;cos] in cols 0:T, [-sin;sin] in cols T:2T (gpsimd queue: keeps
        # the sync queue clear for hid + first weight tiles)
        cs_sb = cs_pool.tile([128, 2 * T], F32)
        nc.gpsimd.dma_start(cs_sb, csd)

        ones_col = const.tile([128, 128], F32)        # fp32 ones (norm reduce)
        nc.vector.memset(ones_col, 1.0)
        ones_bf = const.tile([128, 128], BF16)        # bf16 ones (denominator)
        nc.vector.memset(ones_bf, 1.0)
        bqk_sb = const.tile([128, 2 * H], F32)
        nc.sync.dma_start(bqk_sb, bqk)
        gqk_sb = const.tile([128, 2 * H], F32)
        nc.sync.dma_start(gqk_sb, gqk)
        bgqk_sb = const.tile([128, 2 * H], F32)
        nc.sync.dma_start(bgqk_sb, bgqk)
        bo_sb = const.tile([128, DIM // 128], F32)
        nc.sync.dma_start(bo_sb, bo)
        eps_q = const.tile([128, 1], F32)
        nc.vector.memset(eps_q, D * EPS)
        eps_k = const.tile([128, 1], F32)
        nc.vector.memset(eps_k, EPS)

        # ================= QKV projections, norms, rope, gathers =============
        def qk_path(which, dest_tiles, chunk_done=None):
            mlo = H if which == "k" else 0
            # --- projection + g-folded evac + sum of squares ---
            ssq = small_pool.tile([128, T], F32, tag="ssq")
            works = []
            for mi in range(H):
                m = mlo + mi
                work = wrk_pool.tile([128, T], BF16, tag=f"work{mi}",
                                     name=f"work{mi}")
                works.append(work)
                tsq = tmp_pool.tile([128, T], F32, tag="tsq")
                ps2 = big_ps.tile([128, 1024], F32, tag="qkps")
                w_sb = wl_pool.tile([128, KQ * 128], BF16, tag="wqk")
                nc.sync.dma_start(
                    w_sb.rearrange("p (c m) -> p c m", c=KQ),
                    WqkT.tensor[:, 128 * m:128 * (m + 1)].rearrange(
                        "(c p) m -> p c m", p=128))
                for kc in range(KQ):
                    for g in range(2):
                        nc.tensor.matmul(ps2[:, PS2[g]],
                                         w_sb[:, 128 * kc:128 * (kc + 1)],
                                         hid[kc][:, GS[g]],
                                         start=(kc == 0),
                                         stop=(kc == KQ - 1))
                b = bqk_sb[:, m:m + 1]
                # work = g*(x+b) in bf16; squares from (x+b) in fp32
                act2(work, ps2, AF.Identity, bias=bgqk_sb[:, m:m + 1],
                     scale=gqk_sb[:, m:m + 1])
                act2(tsq, ps2, AF.Square, bias=b)
                if mi == 0:
                    nc.vector.tensor_copy(ssq, tsq)
                else:
                    nc.vector.tensor_tensor(ssq, ssq, tsq, ALU.add)
            # --- rms scale, 128-replicated: inv = 1/sqrt(mean+eps) (x 1/sqrt(D)
            # for q), then folded into the rope tables ---
            sq_scale = (D / DIM) if which == "q" else (1.0 / DIM)
            sq_bias = eps_q if which == "q" else eps_k
            inv = small_pool.tile([128, T], F32, tag="inv")
            rt = small_pool.tile([128, T], F32, tag="rt")
            for g in range(2):
                red = red_ps.tile([128, QG], F32, tag=f"dp{g}", name=f"dp{g}")
                nc.tensor.matmul(red, ones_col, ssq[:, GS[g]], start=True,
                                 stop=True)
                nc.scalar.activation(rt[:, GS[g]], red, AF.Sqrt,
                                     bias=sq_bias, scale=sq_scale)
            nc.vector.reciprocal_approx_fast(inv, rt)
            cq = csq_pool.tile([128, T], BF16, tag="cq")
            sq = csq_pool.tile([128, T], BF16, tag="sq")
            nc.vector.tensor_tensor(cq, cs_sb[:, 0:T], inv, ALU.mult)
            nc.vector.tensor_tensor(sq, cs_sb[:, T:2 * T], inv, ALU.mult)
            # --- rope -> bf16 dest, per head chunk ---
            for mi in range(H):
                work = works[mi]
                dest = dest_tiles[mi]
                sw = rope_pool.tile([128, T], BF16, tag="rsw")
                nc.gpsimd.dma_start(sw[0:64, :], work[64:128, :])
                nc.gpsimd.dma_start(sw[64:128, :], work[0:64, :])
                for g in range(2):
                    qs = GS[g]
                    ta = rope_pool.tile([128, QG], BF16, tag="ra")
                    nc.vector.tensor_tensor(ta, work[:, qs], cq[:, qs],
                                            ALU.mult)
                    tb = rope_pool.tile([128, QG], BF16, tag="rb")
                    nc.vector.tensor_tensor(tb, sw[:, qs], sq[:, qs],
                                            ALU.mult)
                    nc.vector.tensor_tensor(dest[:, qs], ta, tb, ALU.add)
                if chunk_done is not None:
                    chunk_done(mi, dest)

        # ---- v: token-major, contraction over dim chunks + bias row ----
        def emit_v_load(og):
            vb = small_pool.tile([1, 512], BF16, tag="vb")
            nc.sync.dma_start(
                vb, WvTa.tensor[DIM:DIM + 1, 512 * og:512 * (og + 1)])
            vw = [vw_pool.tile([128, 512], BF16, tag=f"vw{kc}",
                               name=f"vw{kc}") for kc in range(KQ)]
            for kc in range(KQ):
                nc.sync.dma_start(
                    vw[kc], WvTa.tensor[128 * kc:128 * (kc + 1),
                                        512 * og:512 * (og + 1)])
            return vb, vw

        def emit_v_compute(og, vb, vw):
            for ti, (t0, tn_) in enumerate(TOK_SPLITS):
                # alternate the two norm-reduce banks: double-buffers the V
                # chains without widening the PSUM budget
                ps = red_ps.tile([128, 512], F32, tag=f"dp{ti % 2}")
                for kc in range(KQ):
                    nc.tensor.matmul(ps[0:tn_, :],
                                     hid[kc][:, t0:t0 + tn_],
                                     vw[kc], start=(kc == 0), stop=False)
                nc.tensor.matmul(ps[0:tn_, :], hid_ones[:, t0:t0 + tn_],
                                 vb, start=False, stop=True)
                vsb = tmp_pool.tile([128, 512], BF16, tag="vsb")
                nc.scalar.activation(vsb[0:tn_, :], ps[0:tn_, :],
                                     AF.Identity)
                # scalar queue: dependent stores interleave there with their
                # producing ACTs (in-order queues head-of-line block)
                nc.scalar.dma_start(vcon[og].ap()[t0:t0 + tn_, :],
                                    vsb[0:tn_, :])
            nc.gpsimd.collective_compute(
                "AllGather", ALU.bypass,
                replica_groups=[list(range(NCORES))],
                ins=[vcon[og].ap()], outs=[gv[og].ap()])

        # ---- k first: its rope + per-head AllGather latency is the critical
        # path to the first attention head; V's matmuls fill that window ----
        k_tiles = [att_pool.tile([128, T], BF16, tag=f"att{h}",
                                 name=f"kt{h}") for h in range(H)]

        def k_chunk_done(mi, dest):
            nc.scalar.dma_start(kcon[mi].ap(), dest)
            nc.gpsimd.collective_compute(
                "AllGather", ALU.bypass,
                replica_groups=[list(range(NCORES))],
                ins=[kcon[mi].ap()], outs=[gk[mi].ap()])

        qk_path("k", k_tiles, k_chunk_done)

        emit_v_compute(0, *emit_v_load(0))
        emit_v_compute(1, *emit_v_load(1))
        emit_v_compute(2, *emit_v_load(2))

        # ---- attention kv window prefetch (2 heads deep) ----
        def kv_prefetch(h):
            ksb = kv_pool.tile([128, 4 * T], BF16, tag="ksb")
            for w in range(4):
                nc.gpsimd.dma_start(
                    ksb[:, w * T:(w + 1) * T],
                    gk[h][bass.ds(kregs[w], 128), :])
            ho = 128 * (h % 4)
            vwin = vt_pool.tile([128, 25 * 128], BF16, tag="vwin")
            for w in range(4):
                lo = 780 * w          # window-space start of this block
                s = lo
                while s < lo + 780:
                    off = s % 128
                    if off:
                        n = min(128 - off, lo + 780 - s)
                    else:
                        n = lo + 780 - s
                    blk = s // 128
                    if off == 0 and n >= 128:
                        nb = n // 128
                        nc.gpsimd.dma_start(
                            vwin[:, 128 * blk:128 * (blk + nb)].rearrange(
                                "p (c d) -> p c d", d=128),
                            gv[h // 4][bass.ds(vregs[w] + (s - lo),
                                               128 * nb),
                                       ho:ho + 128].rearrange(
                                           "(c p) d -> p c d", p=128))
                        s += 128 * nb
                    else:
                        n = min(n, 128 - off)
                        nc.gpsimd.dma_start(
                            vwin[off:off + n,
                                 128 * blk:128 * (blk + 1)],
                            gv[h // 4][bass.ds(vregs[w] + (s - lo), n),
                                       ho:ho + 128])
                        s += n
            return ksb, vwin

        # issue heads 0/1 before the q path so attention can start the moment
        # q head 0 is roped (their DMAs depend only on the k/v collectives)
        pref = [kv_prefetch(0), kv_prefetch(1)]

        # ---- q ----
        q_tiles = [q_pool.tile([128, T], BF16, tag=f"q{h}", name=f"qt{h}")
                   for h in range(H)]
        qk_path("q", q_tiles)

        # ================= attention =========================================
        att_tiles = []
        for h in range(H):
            ksb, vwin = pref[h]
            ath = att_pool.tile([128, T], BF16, tag=f"att{h}")
            att_tiles.append(ath)
            op2 = op_ps.tile([128, 1024], F32, tag="op")
            dps = [red_ps.tile([128, QG], F32, tag="dp0", name="dp0"),
                   red_ps.tile([128, QG], F32, tag="dp1", name="dp1")]
            # per-chunk interleave: scores -> exp -> den+PV accumulate
            for ci in range(N_KC):
                c0 = 128 * ci
                cn = min(128, 4 * T - c0)          # window is 3120 tokens
                sp2 = big_ps.tile([128, 1024], F32, tag="qkps")
                for g in range(2):
                    nc.tensor.matmul(
                        sp2[0:cn, PS2[g]], ksb[:, c0:c0 + cn],
                        q_tiles[h][:, GS[g]], start=True, stop=True)
                pr = probs_pool.tile([128, 2 * QG], BF16, tag="pr")
                act2(pr[0:cn, :], sp2[0:cn, :], AF.Exp)
                for g in range(2):
                    nc.tensor.matmul(dps[g], ones_bf[0:cn, :],
                                     pr[0:cn, GS[g]],
                                     start=(ci == 0), stop=(ci == N_KC - 1))
                vt = vwin[:, 128 * ci:128 * (ci + 1)]
                for g in range(2):
                    nc.tensor.matmul(op2[:, PS2[g]], vt[0:cn, :],
                                     pr[0:cn, GS[g]],
                                     start=(ci == 0), stop=(ci == N_KC - 1))
            osb = attm_pool.tile([128, 2 * QG], F32, tag="osb")
            act2(osb, op2, AF.Identity)
            dsb = attm_pool.tile([128, 2 * QG], F32, tag="dsb")
            for g in range(2):
                nc.vector.reciprocal_approx_fast(dsb[:, GS[g]], dps[g])
            nc.vector.tensor_tensor(ath, osb, dsb, ALU.mult)
            if h + 2 < H:
                pref.append(kv_prefetch(h + 2))

        # ================= output projection =================================
        for od in range(DIM // 128):
            wo = wo_pool.tile([128, HD], BF16, tag="wo")
            nc.sync.dma_start(
                wo.rearrange("p (c m) -> p c m", c=H),
                WoT.tensor[:, 128 * od:128 * (od + 1)].rearrange(
                    "(c p) m -> p c m", p=128))
            ot = o_pool.tile([128, T], F32, tag="ot")
            ps2 = big_ps.tile([128, 1024], F32, tag="qkps")
            for hc in range(H):
                for g in range(2):
                    nc.tensor.matmul(ps2[:, PS2[g]],
                                     wo[:, 128 * hc:128 * (hc + 1)],
                                     att_tiles[hc][:, GS[g]],
                                     start=(hc == 0), stop=(hc == H - 1))
            act2(ot, ps2, AF.Identity, bias=bo_sb[:, od:od + 1])
            nc.scalar.dma_start(outT.tensor[128 * od:128 * (od + 1), :], ot)


_CACHED_NC = None
_LAST_IN_MAPS = None


def _get_nc():
    global _CACHED_NC
    if _CACHED_NC is None:
        _CACHED_NC = _build_nc()
    return _CACHED_NC


def _deinterleave(n):
    """Permutation putting even rotary lanes first within each 128-dim head."""
    idx = np.arange(n).reshape(-1, D)
    return np.concatenate([idx[:, 0::2], idx[:, 1::2]], axis=1).reshape(-1)


def kernel(hidden_states, freqs_cos, freqs_sin, W_qkv, b_qkv, gq, gk, W_out,
           b_out):
    hidden_states = np.asarray(hidden_states, dtype=np.float32)
    freqs_cos = np.asarray(freqs_cos, dtype=np.float32)
    freqs_sin = np.asarray(freqs_sin, dtype=np.float32)
    W_qkv = np.asarray(W_qkv, dtype=np.float32)
    b_qkv = np.asarray(b_qkv, dtype=np.float32)
    gq = np.asarray(gq, dtype=np.float32)
    gk = np.asarray(gk, dtype=np.float32)
    W_out = np.asarray(W_out, dtype=np.float32)
    b_out = np.asarray(b_out, dtype=np.float32)

    nc = _get_nc()

    perm = _deinterleave(HD)
    Wq, Wk, Wv = W_qkv[:HD][perm], W_qkv[HD:2 * HD][perm], W_qkv[2 * HD:]
    bq, bk, bv = b_qkv[:HD][perm], b_qkv[HD:2 * HD][perm], b_qkv[2 * HD:]
    gqp, gkp = gq[perm], gk[perm]

    WqkT = np.ascontiguousarray(
        np.concatenate([Wq, Wk], axis=0).T).astype(NP_BF16)   # [1536, 3072]
    WvTa = np.concatenate([Wv.T, bv[None, :]],
                          axis=0).astype(NP_BF16)             # [1537, 1536]
    bcat = np.concatenate([bq, bk])
    gcat = np.concatenate([gqp, gkp])
    bqk_t = np.ascontiguousarray(bcat.reshape(2 * H, 128).T)  # [128, 24]
    gqk_t = np.ascontiguousarray(gcat.reshape(2 * H, 128).T)
    bgqk_t = np.ascontiguousarray((bcat * gcat).reshape(2 * H, 128).T)
    WoT = np.ascontiguousarray(W_out.T).astype(NP_BF16)       # [1536, 1536]
    bo = np.ascontiguousarray(b_out.reshape(DIM // 128, 128).T)  # [128, 12]

    in_maps = []
    for c in range(NCORES):
        sl = slice(c * T, (c + 1) * T)
        hidT = np.concatenate([
            np.ascontiguousarray(hidden_states[0, sl, :].T),
            np.ones((1, T), np.float32)], axis=0).astype(NP_BF16)  # [1537, 780]
        f = (c * T) // L
        if f == 0:
            win = [0, 1, 0, 1]
        else:
            base = 2 * (f - 1)
            win = [base, base + 1, base + 2, base + 3]
        cc = np.ascontiguousarray(freqs_cos[sl].T)            # [64, 780]
        ss = np.ascontiguousarray(freqs_sin[sl].T)
        csd = np.concatenate([
            np.concatenate([cc, cc], axis=0),
            np.concatenate([-ss, ss], axis=0)], axis=1)       # [128, 1560]
        in_maps.append({
            "hidT": hidT,
            "csd": csd,
            "wink": np.asarray([[w * 128 for w in win]], np.uint32),
            "winv": np.asarray([[w * T for w in win]], np.uint32),
            "WqkT": WqkT, "WvTa": WvTa, "bqk": bqk_t, "gqk": gqk_t,
            "bgqk": bgqk_t, "WoT": WoT, "bo": bo,
        })

    global _LAST_IN_MAPS
    _LAST_IN_MAPS = in_maps
    res = bass_utils.run_bass_kernel_spmd(nc, in_maps,
                                          core_ids=list(range(NCORES)))
    out = np.empty((1, S, DIM), np.float32)
    for c in range(NCORES):
        out[0, c * T:(c + 1) * T, :] = res.results[c]["outT"].T
    return out
